# revision 30
# baseline (speedup 1.0000x reference)
"""Trainium2 Bass kernel for nn_CortexBlock_59940563583556.

Math note (exact, not an approximation): the reference initializes the
fast-weight state U0 = V0 = 0 inside reference() itself, and every term
of the scan's update to U/V is proportional to ku = k_t^T @ U (zero when
U == 0).  By induction U_t == V_t == 0 for the whole scan, for ANY input
values.  Hence k_fast == 0, score_fast == 0, and (since mix_logit is
added to both logits, softmax is shift-invariant) the block reduces
exactly to:

    q = h @ Wq.T ; k = h @ Wk.T ; v = h @ Wv.T          (per-head split)
    g[b,t,h]  = sigmoid( sum_d q[b,t,h,d] * k[b,t,h,d] / sqrt(64) )
    out       = (g * v  per head) @ Wo.T

m_gate / alpha_scale / Wa / ba / mix_logit do not affect the output.

Sharding: data-parallel over the 8192 rows of the flattened [B*T, D]
activations across 8 NeuronCores (1024 rows each); weights replicated.

Precision: q and k exist ONLY inside the per-head gate
sigmoid(q.k/8) whose derivative is <= 1/4, so their GEMMs run in
fp8-e4m3 DoubleRow mode (2 contraction rows per PE cell, ~1.8x the
bf16 matmul rate).  Wq/Wk are pre-scaled by 64 on the host (fp8 has
~3.6% quantization noise regardless of scale; x64 keeps the 0.02-std
weights well inside e4m3 normal range) and the 64*64 factor is divided
back out inside the sigmoid's scale.  v/out GEMMs stay bf16.  Host-
simulated end-to-end error of this exact scheme: 1.3e-2 max-abs vs the
2e-2 gate (bf16-only: 3.4e-3).  ml_dtypes float8_e4m3 max-normal (240)
matches TRN FP8_EXP4.

All operand prep happens on the HOST: weights/activations cast and
pre-transposed into final SBUF layouts (contraction on partitions); h
is shipped twice (fp8 for q/k, bf16 for v).

Clocking (measured, decisive): the HAM governor releases the 2.4GHz
clock only after ~3.4us of CONTIGUOUS PE activity right at kernel
start; without it the whole run is pinned at 2.0GHz (+20%).  The
warmup burns 8 back-to-back 512-row matmuls to trigger the release,
with a 1-column stationary so it costs ~1/128 the MAC power of real
matmuls — early full-power density risks a 3.4-6.8us half-clock
"repayment" window from the same governor.  Do not add PE work or
gaps inside the warmup block.  Tried and rejected: delaying the
warmup to absorb early DMA gaps (release fails ~3 in 4, +17us);
16 warmups (adds +-2us variance racing phase A against DMA);
trailing dummy matmuls to keep the boost clock through the epilogue
(broke the release); denser early DMA maps via SWDGE offload
(triggers repayment quanta).  The warmup is necessary but NOT
sufficient: the device must also be in a willing power state — late
in one session the same byte-exact NEFFs (this kernel AND the old
two-pass baseline) all dropped to 2.0GHz globally.  The relative
ranking held in both regimes (this kernel ~1.5-2.5us ahead), so
optimize cycle count and treat the absolute clock as environmental.

PE schedule (ordered so every arriving weight block enables ~2us of
matmuls; the gpsimd SWDGE ring starts ~10us late so nothing phases
A/B need goes there):
  - phase A: q for row tiles 0-3, kt2-outer ACROSS tiles (fp8).
  - phase B: k (fp8) for tiles 0-1 then 2-3, kt2-outer across tiles.
  - phase C: v (bf16) per tile 0-3; gating chains overlap.
  - phase D: tiles 4-7 per tile: q+k fp8 rounds, v bf16 rounds, then
    the out-GEMM of tile t-4 on the remaining PSUM pair (folding the
    old pass 2 into the stream removes its 13us post-matmul tail).
  - phase E: out-GEMMs 4-7; last tile chunk-serial with quarter-width
    PSUM copies (split ACT/DVE) and out-DMAs on scalar+sync queues
    (o4-o6 ch1 on the idle SWDGE ring) so ~1.5us trails the last
    matmul instead of 13us.
PSUM: 8 banks as 4 pairs, manually scheduled (reuse chains in
comments) so write-after-read waits stay off the PE critical path.
"""

import numpy as np
import ml_dtypes

import concourse.bass as bass
import concourse.mybir as mybir
import concourse.tile as tile
from concourse import bacc
from concourse.bass_utils import run_bass_kernel_spmd

F32 = mybir.dt.float32
BF16 = mybir.dt.bfloat16
FP8 = mybir.dt.float8e4

N_CORES = 8
D = 1024          # model dim
ROWS = 8192       # B*T
M_CORE = ROWS // N_CORES   # rows per core
P = 128           # partitions
KT = D // P       # contraction tiles
MT = M_CORE // P  # row tiles per core
NCH = 2           # output-column chunks of 512
CHW = D // NCH    # 512
H = 16            # heads
DH = 64           # head dim
WSCALE = 64.0     # host pre-scale on Wq/Wk before fp8 quantization
INV_SQRT_DH = 1.0 / (DH ** 0.5)

_COMPILED = None
LAST_RESULT = None  # BassKernelResults of the most recent run (for test harness)


def _build():
    nc = bacc.Bacc("TRN2", target_bir_lowering=False, debug=False)

    hT_in = nc.dram_tensor("ht", [KT, P, M_CORE], BF16, kind="ExternalInput")
    h8_in = nc.dram_tensor("h8", [KT, P, M_CORE], FP8, kind="ExternalInput")
    w_in = {
        "wq": nc.dram_tensor("wq", [KT, P, D], FP8, kind="ExternalInput"),
        "wk": nc.dram_tensor("wk", [KT, P, D], FP8, kind="ExternalInput"),
        "wv": nc.dram_tensor("wv", [KT, P, D], BF16, kind="ExternalInput"),
        "wo": nc.dram_tensor("wo", [KT, P, D], BF16, kind="ExternalInput"),
    }
    out = nc.dram_tensor("out", [M_CORE, D], F32, kind="ExternalOutput")

    with tile.TileContext(nc) as tc:
        with (
            tc.tile_pool(name="res", bufs=1) as res_pool,
            tc.tile_pool(name="qsb", bufs=4) as q_pool,
            tc.tile_pool(name="sp", bufs=2) as sp_pool,
            tc.tile_pool(name="small", bufs=4) as small_pool,
            tc.tile_pool(name="y", bufs=2) as y_pool,
            tc.tile_pool(name="yT", bufs=MT) as yT_pool,
            tc.tile_pool(name="osb", bufs=2) as o_pool,
            tc.tile_pool(name="ps", bufs=1, space="PSUM") as ps_pool,
        ):
            # ---- resident operands, host-prepped layouts ----
            wsb = {
                name: res_pool.tile([P, KT, D], FP8 if name in ("wq", "wk")
                                    else BF16, tag=f"w_{name}", name=f"w_{name}")
                for name in ("wq", "wk", "wv", "wo")
            }
            hsb = res_pool.tile([P, KT, M_CORE], BF16, tag="h", name="h")
            h8sb = res_pool.tile([P, KT, M_CORE], FP8, tag="h8", name="h8")

            def tr(ap):
                return ap.rearrange("a p m -> p a m")

            # DMA schedule, ordered by first-need time.  Nothing phase A
            # or B needs goes on gpsimd (SWDGE starts ~10us late).
            #   scalar: wq[0:2] wq[4:6] | wv 0..7 | wo[0:4] | out chunks
            #   sync: h8[0:4]a wq[2:4] wq[6:8] h8[4:8]a wk[0:4] wk[4:8]
            #         hbf[0:4]a hbf[4:8]a | wo[4:8] | yT transposes
            #   gpsimd: hbf cols 512:1024, h8 cols 512:1024 (tiles 4-7)
            A_COLS = 4 * P  # columns (rows of h) used by tiles 0-3
            # phase-A round r (kt2=2r) needs wq[2r:2r+2] AND h8[2r:2r+2]
            # cols 0:512; the pair-blocks zig-zag across both rings in
            # exactly that order so no round waits behind a later block.
            # v-GEMMs are deferred past the k phases, so wv/hbf/wo can
            # stream at leisure after the small fp8 blocks.
            nc.scalar.dma_start(out=wsb["wq"][:, 0:2, :], in_=tr(w_in["wq"][0:2]))
            nc.sync.dma_start(out=h8sb[:, 0:2, 0:A_COLS],
                              in_=tr(h8_in[0:2, :, 0:A_COLS]))
            nc.sync.dma_start(out=wsb["wq"][:, 2:4, :], in_=tr(w_in["wq"][2:4]))
            nc.scalar.dma_start(out=h8sb[:, 2:4, 0:A_COLS],
                              in_=tr(h8_in[2:4, :, 0:A_COLS]))
            nc.scalar.dma_start(out=wsb["wq"][:, 4:6, :], in_=tr(w_in["wq"][4:6]))
            nc.sync.dma_start(out=h8sb[:, 4:6, 0:A_COLS],
                              in_=tr(h8_in[4:6, :, 0:A_COLS]))
            nc.sync.dma_start(out=wsb["wq"][:, 6:8, :], in_=tr(w_in["wq"][6:8]))
            nc.scalar.dma_start(out=h8sb[:, 6:8, 0:A_COLS],
                              in_=tr(h8_in[6:8, :, 0:A_COLS]))
            nc.sync.dma_start(out=wsb["wk"][:, 0:4, :], in_=tr(w_in["wk"][0:4]))
            nc.scalar.dma_start(out=wsb["wk"][:, 4:8, :], in_=tr(w_in["wk"][4:8]))
            nc.scalar.dma_start(out=wsb["wv"][:, 0:4, :], in_=tr(w_in["wv"][0:4]))
            nc.sync.dma_start(out=hsb[:, 0:4, 0:A_COLS],
                              in_=tr(hT_in[0:4, :, 0:A_COLS]))
            nc.sync.dma_start(out=wsb["wv"][:, 4:8, :], in_=tr(w_in["wv"][4:8]))
            nc.scalar.dma_start(out=hsb[:, 4:8, 0:A_COLS],
                              in_=tr(hT_in[4:8, :, 0:A_COLS]))
            nc.scalar.dma_start(out=wsb["wo"][:, 0:4, :], in_=tr(w_in["wo"][0:4]))
            nc.sync.dma_start(out=wsb["wo"][:, 4:8, :], in_=tr(w_in["wo"][4:8]))
            for a in range(0, KT, 4):
                nc.gpsimd.dma_start(out=h8sb[:, a:a + 4, A_COLS:],
                                    in_=tr(h8_in[a:a + 4, :, A_COLS:]))
            for a in range(0, KT, 4):
                nc.gpsimd.dma_start(out=hsb[:, a:a + 4, A_COLS:],
                                    in_=tr(hT_in[a:a + 4, :, A_COLS:]))

            # 8 PSUM banks as 4 pairs of [128, 512] f32 tiles.
            def ps_pair(j):
                return [ps_pool.tile([P, CHW], F32, tag=f"T{2 * j + jo}",
                                     name=f"T{2 * j + jo}")
                        for jo in range(NCH)]

            def jsl(jo):
                return slice(jo * CHW, (jo + 1) * CHW)

            def v_mm(ps_t, i, kt, jo):
                nc.tensor.matmul(
                    out=ps_t,
                    lhsT=hsb[:, kt, i * P:(i + 1) * P],
                    rhs=wsb["wv"][:, kt, jsl(jo)],
                    start=(kt == 0),
                    stop=(kt == KT - 1),
                )

            def qk_mm(ps_t, wname, i, kt2, jo):
                # fp8 DoubleRow: both operands carry 2 contraction tiles.
                nc.tensor.matmul(
                    out=ps_t,
                    lhsT=h8sb[:, kt2:kt2 + 2, i * P:(i + 1) * P],
                    rhs=wsb[wname][:, kt2:kt2 + 2, jsl(jo)],
                    start=(kt2 == 0),
                    stop=(kt2 == KT - 2),
                    perf_mode=mybir.MatmulPerfMode.DoubleRow,
                )

            def q_copies(qp):
                # stage q in SBUF (bf16) to free its banks; the s-mul
                # needs q in SBUF anyway (DVE reads one PSUM operand).
                qsb = []
                for jo in range(NCH):
                    t_ = q_pool.tile([P, CHW], BF16, tag=f"qsb{jo}",
                                     name=f"qsb{jo}")
                    nc.scalar.copy(out=t_, in_=qp[jo])
                    qsb.append(t_)
                return qsb

            yT_tiles = []

            def chain(qsb, kp, vp):
                # s[m,h] = sum_{d in head} q*k ; g = sigmoid(s * scale)
                # (scale folds away the fp8 WSCALE^2) ; y = g*v (bf16) ;
                # yT via DMA transpose.  All DVE except the sigmoid; sp
                # in bf16 for the 2x reduce read rate.
                sp = sp_pool.tile([P, D], BF16, tag="sp", name="sp")
                for jo in range(NCH):
                    nc.vector.tensor_mul(out=sp[:, jsl(jo)], in0=qsb[jo],
                                         in1=kp[jo])
                s = small_pool.tile([P, H], F32, tag="s", name="s")
                nc.vector.reduce_sum(
                    out=s,
                    in_=sp.rearrange("p (h d) -> p h d", d=DH),
                    axis=mybir.AxisListType.X,
                )
                g = small_pool.tile([P, H], F32, tag="g", name="g")
                nc.scalar.activation(
                    out=g, in_=s,
                    func=mybir.ActivationFunctionType.Sigmoid,
                    scale=INV_SQRT_DH / (WSCALE * WSCALE),
                )
                y = y_pool.tile([P, D], BF16, tag="y", name="y")
                for jo in range(NCH):
                    g_sl = g[:, jo * (H // NCH):(jo + 1) * (H // NCH)]
                    g_bc = bass.AP(
                        tensor=g_sl.tensor, offset=g_sl.offset,
                        ap=[*g_sl.ap, [0, DH]],
                    )
                    nc.vector.tensor_mul(
                        out=y[:, jsl(jo)].rearrange("p (h d) -> p h d", d=DH),
                        in0=vp[jo].rearrange("p (h d) -> p h d", d=DH),
                        in1=g_bc,
                    )
                yT = yT_pool.tile([P, KT, P], BF16, tag="yT", name="yT")
                nc.sync.dma_start_transpose(out=yT, in_=y)
                yT_tiles.append(yT)

            # ---- PE warm-up during the initial DMA wait ----
            # The HAM clock gate starts the PE at 1.2 GHz and needs
            # ~3.4us of sustained activity to release to 2.4 GHz.  The
            # PE is data-starved for ~4us while the first weight blocks
            # stream in, so burn that window on dependency-free matmuls
            # over (uninitialized) SBUF scratch into a bank phase A
            # overwrites (start=True clears it).  8 cold N=512 matmuls
            # ~= 3.4us.
            scratch = res_pool.tile([P, CHW], BF16, tag="warm", name="warm")
            nc.vector.memset(scratch, 0.0)
            warm_ps = ps_pair(0)
            for _ in range(8):
                nc.tensor.matmul(out=warm_ps[0][0:1, :], lhsT=scratch[:, 0:1],
                                 rhs=scratch, start=True, stop=True)

            # ---- phase A: q for tiles 0-3, kt2-outer ACROSS tiles ----
            qA = [ps_pair(t) for t in range(4)]
            for kt2 in range(0, KT, 2):
                for t in range(4):
                    for jo in range(NCH):
                        qk_mm(qA[t][jo], "wq", t, kt2, jo)
            # copy order t0,t2,t1,t3 matches the order B touches banks
            qsbA = [None] * 4
            for t in (0, 2, 1, 3):
                qsbA[t] = q_copies(qA[t])

            # ---- phase B: k (fp8) for tiles 0-1 then 2-3 ----
            # k[t] -> PR[t] (ex qA[t], freed by the qsb copies above).
            kB = [ps_pair(t) for t in range(4)]
            for kt2 in range(0, KT, 2):
                for t in range(2):
                    for jo in range(NCH):
                        qk_mm(kB[t][jo], "wk", t, kt2, jo)
            for kt2 in range(0, KT, 2):
                for t in range(2, 4):
                    for jo in range(NCH):
                        qk_mm(kB[t][jo], "wk", t, kt2, jo)

            # ---- phase C: v (bf16) per tile 0-3, sequential ----
            # v[t] -> PR[t] (ex k[t], freed by tile t's sp-muls, which
            # need only qsb+k); each tile's gating chain overlaps the
            # next tile's v rounds.
            vC = []
            for t in range(4):
                vp = ps_pair(t)
                vC.append(vp)
                for kt in range(KT):
                    for jo in range(NCH):
                        v_mm(vp[jo], t, kt, jo)
                chain(qsbA[t], kB[t], vp)

            def out_gemm(i, op, chunk_serial=False):
                # out tile i = yT_i @ wo, accumulated over kt into op[0/1].
                if chunk_serial:
                    orders = [(kt, jo) for jo in range(NCH) for kt in range(KT)]
                else:
                    orders = [(kt, jo) for kt in range(KT) for jo in range(NCH)]
                for kt, jo in orders:
                    nc.tensor.matmul(
                        out=op[jo],
                        lhsT=yT_tiles[i][:, kt, :],
                        rhs=wsb["wo"][:, kt, jsl(jo)],
                        start=(kt == 0),
                        stop=(kt == KT - 1),
                    )

            def out_drain(i, op, nsplit=2):
                # PSUM -> SBUF copies split ACT/DVE; DMA pieces on BOTH
                # HWDGE queues (ACT piece -> scalar queue, DVE piece ->
                # sync queue) so the final drain halves.
                ms = slice(i * P, (i + 1) * P)
                osb = o_pool.tile([P, D], F32, tag="osb", name="osb")
                w_ = D // nsplit
                per = CHW // w_  # pieces per 512-col PSUM chunk: 1 or 2
                for jo in range(nsplit):
                    sl = slice(jo * w_, (jo + 1) * w_)
                    src = op[jo // per][:, (jo % per) * w_:(jo % per) * w_ + w_]
                    if jo % 2 == 0:
                        nc.scalar.copy(out=osb[:, sl], in_=src)
                        nc.scalar.dma_start(out=out[ms, sl], in_=osb[:, sl])
                    else:
                        nc.vector.tensor_copy(out=osb[:, sl], in_=src)
                        # mid-E ch1 outs ride the idle SWDGE ring so the
                        # sync queue is empty for the last tile's pieces
                        eng = nc.gpsimd if (nsplit == 2 and 4 <= i < MT - 1)                             else nc.sync
                        eng.dma_start(out=out[ms, sl], in_=osb[:, sl])

            # ---- phase D: tiles 4-7: q+k fp8 rounds, v rounds, then the
            # out-GEMM of tile t-4 on the remaining PSUM pair (folding
            # pass 2 into the stream kills the baseline's 13us tail) ----
            # t4: q PR0 (ex k2, freed by t2 sp-muls), k PR1 (ex k3),
            # v PR2 (ex v2, t2 y-muls), o PR3 (ex v3); rotate -1 each tile.
            for ti, t in enumerate(range(4, MT)):
                jq, jk, jv = (-ti) % 4, (1 - ti) % 4, (2 - ti) % 4
                jf = (3 - ti) % 4  # the pair not used by q/k/v this tile
                qp, kp, vp = ps_pair(jq), ps_pair(jk), ps_pair(jv)
                for kt2 in range(0, KT, 2):
                    for ps_t, wname, jo in ((qp[0], "wq", 0), (qp[1], "wq", 1),
                                            (kp[0], "wk", 0), (kp[1], "wk", 1)):
                        qk_mm(ps_t, wname, t, kt2, jo)
                for kt in range(KT):
                    for jo in range(NCH):
                        v_mm(vp[jo], t, kt, jo)
                op = ps_pair(jf)
                out_gemm(t - 4, op)
                chain(q_copies(qp), kp, vp)
                out_drain(t - 4, op)

            # ---- phase E: out-GEMMs 4-7; last tile chunk-serial with
            # quarter-width drain so only ~1us trails the final matmul ----
            for ti, i in enumerate(range(4, MT)):
                # pair0 just held o3 (drained ~1us after D ends); start E
                # from pair1 (ex q7, freed during t7's v rounds).
                op = ps_pair((ti + 1) % 4)
                last = (i == MT - 1)
                out_gemm(i, op, chunk_serial=last)
                out_drain(i, op, nsplit=4 if last else 2)

            # ~1.3us of trailing 1-column matmuls: PE activity spans the
            # final drain, so the HAM boost window (which closes ~2.5-5us
            # after the last PE activity) covers the drain and the first
            # quanta of the teardown epilogue at full clock.
            hold_ps = ps_pair(1)
            for _ in range(11):
                nc.tensor.matmul(out=hold_ps[0][0:1, :], lhsT=scratch[:, 0:1],
                                 rhs=scratch, start=True, stop=True)

    nc.compile()
    return nc


def kernel(hidden_states, m_gate, alpha_scale, Wq, Wk, Wv, Wo, Wa, ba, mix_logit,
           **_unused):
    global _COMPILED, LAST_RESULT
    if _COMPILED is None:
        _COMPILED = _build()
    nc = _COMPILED

    bf16 = ml_dtypes.bfloat16
    fp8 = ml_dtypes.float8_e4m3  # IEEE-style: max 240, matches TRN FP8_EXP4
    h = np.asarray(hidden_states, dtype=np.float32).reshape(ROWS, D)

    def prep_w(w, dtype, scale=1.0):
        # W [j, d] -> W^T [kt, p, j]: wT[kt, p, j] = W[j, kt*128+p]
        wt = np.ascontiguousarray(np.asarray(w, dtype=np.float32).T * scale)
        return wt.reshape(KT, P, D).astype(dtype)

    wq = prep_w(Wq, fp8, WSCALE)
    wk = prep_w(Wk, fp8, WSCALE)
    wv = prep_w(Wv, bf16)
    wo = prep_w(Wo, bf16)

    in_maps = []
    for c in range(N_CORES):
        hc = h[c * M_CORE:(c + 1) * M_CORE]  # [M_CORE, D]
        # hT [kt, p, m] = h[m, kt*128+p]
        ht = np.ascontiguousarray(hc.T.reshape(KT, P, M_CORE))
        in_maps.append({
            "ht": ht.astype(bf16), "h8": ht.astype(fp8),
            "wq": wq, "wk": wk, "wv": wv, "wo": wo,
        })

    res = run_bass_kernel_spmd(nc, in_maps, core_ids=list(range(N_CORES)))
    LAST_RESULT = res
    out = np.concatenate([res.results[c]["out"] for c in range(N_CORES)], axis=0)
    B, T = 4, 2048
    return out.reshape(B, T, D)

